# revision 34
# baseline (speedup 1.0000x reference)
"""Trainium2 Bass kernel for a cross-attention layer.

Reference computation (per batch b):
    vision = inputs[b, :, :1024]; text = inputs[b, :, 1024:]
    Q = vision @ Wq.T + bq;  K = text @ Wk.T + bk;  V = text @ Wv.T + bv
    attn = softmax(Q @ K.T / 32, axis=-1)                 # [S, S]
    cav  = attn @ V                                       # [S, 1024]
    cat  = attn.T @ vision                                # [S, 1024]

Sharding: 8 cores = 4 batches x 2 query-halves (1024 q rows each).
Each core computes K/V for the full 2048-key sequence of its batch
(replicated across the pair), its q-half of attn and cav, and a partial
cat (summed over the pair on the host).

Device algorithm (per core, streaming k in 4 slabs of 512):
  - QT[e,q] = Wq @ visionT (+bq)       K-transposed layouts come straight
  - KT[e,k] = Wk @ textT   (+bk)       out of the PE (out = lhsT.T @ rhs)
  - V[k,e]  = textT.T @ WvT (+bv)
  - exp_s[q,k]  = exp(QT.T @ KT / 32)  (no max subtraction: scores~N(0,1))
  - expT[k,q]   = exp(KT.T @ QT / 32)  (recomputed transposed - cheaper
                                        than PE-transposing attn)
  - Z[q] += row-sums of exp_s (fused into the exp activation)
  - cav_acc[q,e] += expT.T @ V         (PSUM -> SBUF accumulation)
  - spill exp_s slab to DRAM
  - finally: cav = cav_acc * (1/Z[q]);
    cat[k,d] = sum_q exp_s[q,k] * (vision[q,d]/Z[q])  (second pass over
    the spilled exp_s; 1/Z folded into a pre-scaled vision copy)
"""

import numpy as np
import ml_dtypes

B, S, D = 4, 2048, 1024
QH = 1024          # query rows per core
NSLAB = 4
KSLAB = 512
NCORES = 8

_CACHE = {}


def _build(reps=1, use_cc=True):
    import contextlib

    import concourse.mybir as mybir
    from concourse import bacc
    from concourse.masks import make_identity
    from concourse.tile import TileContext

    DT = mybir.dt.float16  # fp16: same PE rate as bf16, 8x lower rounding err
    F32 = mybir.dt.float32
    AF = mybir.ActivationFunctionType
    ADD = mybir.AluOpType.add
    SCALE = float(1.0 / np.sqrt(np.float32(D)))

    nc = bacc.Bacc()
    visionT = nc.dram_tensor("visionT", [D, QH], DT, kind="ExternalInput")
    vision = nc.dram_tensor("vision", [QH, D], DT, kind="ExternalInput")
    # own half of textT: columns [h*1024, (h+1)*1024) of the full [D, S]
    # (without collectives: the full textT, both halves computed locally)
    tw = S // 2 if use_cc else S
    textTo = nc.dram_tensor("textTo", [D, tw], DT, kind="ExternalInput")
    wqT = nc.dram_tensor("wqT", [D, D], DT, kind="ExternalInput")
    wkT = nc.dram_tensor("wkT", [D, D], DT, kind="ExternalInput")
    wvT = nc.dram_tensor("wvT", [D, D], DT, kind="ExternalInput")
    bqp = nc.dram_tensor("bqp", [128, 8], F32, kind="ExternalInput")
    bkp = nc.dram_tensor("bkp", [128, 8], F32, kind="ExternalInput")
    bvr = nc.dram_tensor("bvr", [1, D], DT, kind="ExternalInput")
    cav_o = nc.dram_tensor("cav", [QH, D], F32, kind="ExternalOutput")
    cat_o = nc.dram_tensor("catp", [S, D], F32, kind="ExternalOutput")
    spill = nc.dram_tensor("spill", [NSLAB, 128, 8, KSLAB], DT)
    # pair-wise K/V exchange: [KT_own | V_own] -> allgathered [4, 1024, 1024]
    kv_own = nc.dram_tensor("kv_own", [2, D, D], DT)
    kv_sh = nc.dram_tensor("kv_sh", [4, D, D], DT)

    visionT_r = visionT.rearrange("(dt p) q -> p dt q", p=128)
    vision_r = vision.rearrange("(qt p) d -> p qt d", p=128)
    textTo_r = textTo.rearrange("(dt p) k -> p dt k", p=128)
    wq_r = wqT.rearrange("(dt p) e -> p dt e", p=128)
    wk_r = wkT.rearrange("(dt p) e -> p dt e", p=128)
    wv_r = wvT.rearrange("(dt p) e -> p dt e", p=128)
    cav_r = cav_o.rearrange("(qt p) e -> p qt e", p=128)
    cat_r = cat_o.rearrange("(kt p) d -> p kt d", p=128)

    with TileContext(nc) as tc:
        rep_cm = tc.For_i(0, reps, 1) if reps > 1 else contextlib.nullcontext()
        with (
            rep_cm,
            tc.tile_pool(name="const", bufs=1) as const,
            tc.tile_pool(name="acc", bufs=1) as accp,
            tc.tile_pool(name="stats", bufs=1) as stats,
            tc.tile_pool(name="mmps", bufs=4, space="PSUM") as mmps,
            tc.tile_pool(name="cavps", bufs=2, space="PSUM") as cavps,
        ):
            bq_sb = const.tile([128, 8], F32)
            nc.sync.dma_start(out=bq_sb, in_=bqp[:])
            bk_sb = const.tile([128, 8], F32)
            nc.sync.dma_start(out=bk_sb, in_=bkp[:])
            bv_bc = const.tile([128, D], DT)
            nc.sync.dma_start(out=bv_bc, in_=bvr[:].to_broadcast((128, D)))
            ident = const.tile([128, 128], DT)
            make_identity(nc, ident)
            qt_sb = const.tile([128, 8, QH], DT)
            cav_acc = accp.tile([128, 8, D], F32)
            z_acc = stats.tile([128, 8], F32)
            invz = stats.tile([128, 8], F32)
            if not use_cc:
                # full KT/V stay resident in SBUF (no DRAM round-trip)
                kt_full = accp.tile([128, 8, S], DT)
                v_full = accp.tile([128, 16, D], DT)

            # ---- Phase A: KT/V projections (own half + pair AllGather when
            # use_cc; both halves locally otherwise) ----
            nhalves = 1 if use_cc else 2
            with tc.tile_pool(name="phA", bufs=1) as phA:
                wk_sb = phA.tile([128, 8, D], DT)
                nc.sync.dma_start(out=wk_sb, in_=wk_r)
                wv_sb = phA.tile([128, 8, D], DT)
                nc.sync.dma_start(out=wv_sb, in_=wv_r)
                for hh in range(nhalves):
                    tTo = phA.tile([128, 8, D], DT, tag="tTo", bufs=2)
                    nc.sync.dma_start(
                        out=tTo, in_=textTo_r[:, :, hh * D:(hh + 1) * D]
                    )
                    if use_cc:
                        ktow = phA.tile([128, 8, D], DT, tag="ktow")
                        vow = phA.tile([128, 8, D], DT, tag="vow")
                    else:
                        ktow = kt_full[:, :, hh * D:(hh + 1) * D]
                        vow = v_full[:, 8 * hh:8 * (hh + 1), :]
                    for et in range(8):
                        for kc in range(2):
                            ps = mmps.tile([128, 512], F32, tag="mm")
                            for dt in range(8):
                                nc.tensor.matmul(
                                    ps,
                                    lhsT=wk_sb[:, dt, et * 128:(et + 1) * 128],
                                    rhs=tTo[:, dt, kc * 512:(kc + 1) * 512],
                                    start=(dt == 0),
                                    stop=(dt == 7),
                                )
                            nc.scalar.activation(
                                out=ktow[:, et, kc * 512:(kc + 1) * 512],
                                in_=ps,
                                func=AF.Identity,
                                bias=bk_sb[:, et:et + 1],
                                scale=1.0,
                            )
                    for kst in range(8):
                        for ec in range(2):
                            ps = mmps.tile([128, 512], F32, tag="mm")
                            for dt in range(8):
                                nc.tensor.matmul(
                                    ps,
                                    lhsT=tTo[:, dt, kst * 128:(kst + 1) * 128],
                                    rhs=wv_sb[:, dt, ec * 512:(ec + 1) * 512],
                                    start=(dt == 0),
                                    stop=(dt == 7),
                                )
                            nc.vector.tensor_tensor(
                                out=vow[:, kst, ec * 512:(ec + 1) * 512],
                                in0=ps,
                                in1=bv_bc[:, ec * 512:(ec + 1) * 512],
                                op=ADD,
                            )
                    if use_cc:
                        nc.sync.dma_start(
                            out=kv_own[0].rearrange("(et p) k -> p et k", p=128),
                            in_=ktow,
                        )
                        nc.sync.dma_start(
                            out=kv_own[1].rearrange("(kt p) e -> p kt e", p=128),
                            in_=vow,
                        )
                if use_cc:
                    nc.gpsimd.collective_compute(
                        "AllGather",
                        mybir.AluOpType.bypass,
                        replica_groups=[[2 * i, 2 * i + 1] for i in range(4)],
                        ins=[kv_own[:]],
                        outs=[kv_sh[:]],
                    )

            # ---- Phase 0: QT[e,q] = Wq @ visionT + bq ----
            with tc.tile_pool(name="ph0", bufs=1) as ph0:
                vT_sb = ph0.tile([128, 8, QH], DT)
                nc.sync.dma_start(out=vT_sb, in_=visionT_r)
                wq_sb = ph0.tile([128, 8, D], DT)
                nc.sync.dma_start(out=wq_sb, in_=wq_r)
                for et in range(8):
                    for qc in range(2):
                        ps = mmps.tile([128, 512], F32, tag="mm")
                        for dt in range(8):
                            nc.tensor.matmul(
                                ps,
                                lhsT=wq_sb[:, dt, et * 128:(et + 1) * 128],
                                rhs=vT_sb[:, dt, qc * 512:(qc + 1) * 512],
                                start=(dt == 0),
                                stop=(dt == 7),
                            )
                        nc.scalar.activation(
                            out=qt_sb[:, et, qc * 512:(qc + 1) * 512],
                            in_=ps,
                            func=AF.Identity,
                            bias=bq_sb[:, et:et + 1],
                            scale=1.0,
                        )

            # ---- Phase 1: stream k slabs ----
            with (
                tc.tile_pool(name="slab1", bufs=1) as sl1,
                tc.tile_pool(name="slab2", bufs=2) as sl2,
            ):
                for s in range(NSLAB):
                    hs = s // 2            # which pair-half owns this slab
                    k0l = (s % 2) * KSLAB  # offset within that half

                    if use_cc:
                        # KT[e, k-slab] / V[k-slab, e] from allgathered halves
                        kt_sb = sl2.tile([128, 8, KSLAB], DT, tag="kt")
                        nc.sync.dma_start(
                            out=kt_sb,
                            in_=kv_sh[2 * hs].rearrange(
                                "(et p) k -> p et k", p=128
                            )[:, :, k0l:k0l + KSLAB],
                        )
                        v_sb = sl2.tile([128, 4, D], DT, tag="v")
                        nc.sync.dma_start(
                            out=v_sb,
                            in_=kv_sh[2 * hs + 1].rearrange(
                                "(kt p) e -> p kt e", p=128
                            )[:, 4 * (s % 2):4 * (s % 2) + 4, :],
                        )
                    else:
                        kt_sb = kt_full[:, :, s * KSLAB:(s + 1) * KSLAB]
                        v_sb = v_full[:, 4 * s:4 * (s + 1), :]

                    # scores -> exp_s [q, k-slab], Z row-sum accumulation
                    exps = sl2.tile([128, 8, KSLAB], DT, tag="exps")
                    for qt in range(8):
                        ps = mmps.tile([128, 512], F32, tag="mm")
                        for et in range(8):
                            nc.tensor.matmul(
                                ps,
                                lhsT=qt_sb[:, et, qt * 128:(qt + 1) * 128],
                                rhs=kt_sb[:, et, :],
                                start=(et == 0),
                                stop=(et == 7),
                            )
                        zp = sl1.tile([128, 1], F32, tag="zp", bufs=4)
                        nc.scalar.activation(
                            out=exps[:, qt, :],
                            in_=ps,
                            func=AF.Exp,
                            scale=SCALE,
                            accum_out=zp,
                        )
                        if s == 0:
                            nc.vector.tensor_copy(out=z_acc[:, qt:qt + 1], in_=zp)
                        else:
                            nc.vector.tensor_add(
                                out=z_acc[:, qt:qt + 1],
                                in0=z_acc[:, qt:qt + 1],
                                in1=zp,
                            )
                    nc.sync.dma_start(out=spill[s], in_=exps)

                    # expT [k-slab, q] via PE transpose of exp_s chunks
                    expt = sl1.tile([128, 4, QH], DT, tag="expt", bufs=2)
                    for kst in range(4):
                        for qt in range(8):
                            pst = mmps.tile([128, 128], DT, tag="tr", bufs=2)
                            nc.tensor.transpose(
                                out=pst,
                                in_=exps[:, qt, kst * 128:(kst + 1) * 128],
                                identity=ident,
                            )
                            nc.vector.tensor_copy(
                                out=expt[:, kst, qt * 128:(qt + 1) * 128],
                                in_=pst,
                            )

                    # cav_acc[q, e] += expT.T @ V
                    for qt in range(8):
                        for ec in range(2):
                            ps = cavps.tile([128, 512], F32, tag="cav")
                            for kst in range(4):
                                nc.tensor.matmul(
                                    ps,
                                    lhsT=expt[:, kst, qt * 128:(qt + 1) * 128],
                                    rhs=v_sb[:, kst, ec * 512:(ec + 1) * 512],
                                    start=(kst == 0),
                                    stop=(kst == 3),
                                )
                            if s == 0:
                                nc.vector.tensor_copy(
                                    out=cav_acc[:, qt, ec * 512:(ec + 1) * 512],
                                    in_=ps,
                                )
                            else:
                                nc.vector.tensor_add(
                                    out=cav_acc[:, qt, ec * 512:(ec + 1) * 512],
                                    in0=cav_acc[:, qt, ec * 512:(ec + 1) * 512],
                                    in1=ps,
                                )

            # ---- Phase 2: normalize cav, second pass for cat ----
            nc.vector.reciprocal(out=invz, in_=z_acc)
            with (
                tc.tile_pool(name="ph2", bufs=1) as ph2,
                tc.tile_pool(name="ph2b", bufs=2) as ph2b,
            ):
                for qt in range(8):
                    cav_out = ph2b.tile([128, D], F32, tag="cavo")
                    nc.scalar.activation(
                        out=cav_out,
                        in_=cav_acc[:, qt, :],
                        func=AF.Copy,
                        scale=invz[:, qt:qt + 1],
                    )
                    nc.sync.dma_start(out=cav_r[:, qt, :], in_=cav_out)

                vis_sb = ph2.tile([128, 8, D], DT)
                nc.sync.dma_start(out=vis_sb, in_=vision_r)
                vis_sc = ph2.tile([128, 8, D], DT)
                for qt in range(8):
                    nc.scalar.activation(
                        out=vis_sc[:, qt, :],
                        in_=vis_sb[:, qt, :],
                        func=AF.Copy,
                        scale=invz[:, qt:qt + 1],
                    )

                for s in range(NSLAB):
                    rel = ph2b.tile([128, 8, KSLAB], DT, tag="rel")
                    nc.sync.dma_start(out=rel, in_=spill[s])
                    for kst in range(4):
                        cat_sb = ph2b.tile([128, D], F32, tag="cato")
                        for dc in range(2):
                            ps = mmps.tile([128, 512], F32, tag="mm")
                            for qt in range(8):
                                nc.tensor.matmul(
                                    ps,
                                    lhsT=rel[:, qt, kst * 128:(kst + 1) * 128],
                                    rhs=vis_sc[:, qt, dc * 512:(dc + 1) * 512],
                                    start=(qt == 0),
                                    stop=(qt == 7),
                                )
                            nc.vector.tensor_copy(
                                out=cat_sb[:, dc * 512:(dc + 1) * 512], in_=ps
                            )
                        nc.sync.dma_start(out=cat_r[:, s * 4 + kst, :], in_=cat_sb)
    nc.compile()
    return nc


def _get_nc(reps=1, use_cc=True):
    key = ("nc", reps, use_cc)
    if key not in _CACHE:
        _CACHE[key] = _build(reps, use_cc)
    return _CACHE[key]


def _prep_in_maps(inputs, Wq, bq, Wk, bk, Wv, bv, use_cc=True):
    bf = np.float16
    x = np.asarray(inputs, np.float32)
    wqT = np.ascontiguousarray(np.asarray(Wq, np.float32).T.astype(bf))
    wkT = np.ascontiguousarray(np.asarray(Wk, np.float32).T.astype(bf))
    wvT = np.ascontiguousarray(np.asarray(Wv, np.float32).T.astype(bf))
    bqp = np.ascontiguousarray(np.asarray(bq, np.float32).reshape(8, 128).T)
    bkp = np.ascontiguousarray(np.asarray(bk, np.float32).reshape(8, 128).T)
    bvr = np.asarray(bv, np.float32).astype(bf).reshape(1, D)
    in_maps = []
    for c in range(NCORES):
        b, h = divmod(c, 2)
        vis = x[b, :, :D]
        txt = x[b, :, D:]
        visc = vis[h * QH:(h + 1) * QH]
        in_maps.append({
            "visionT": np.ascontiguousarray(visc.T.astype(bf)),
            "vision": np.ascontiguousarray(visc.astype(bf)),
            "textTo": np.ascontiguousarray(
                (txt[h * QH:(h + 1) * QH] if use_cc else txt).T.astype(bf)
            ),
            "wqT": wqT, "wkT": wkT, "wvT": wvT,
            "bqp": bqp, "bkp": bkp, "bvr": bvr,
        })
    return in_maps


def run_on_device(in_maps, trace=False, reps=1, use_cc=True):
    from concourse.bass_utils import run_bass_kernel_spmd

    nc = _get_nc(reps, use_cc)
    return run_bass_kernel_spmd(
        nc, in_maps, core_ids=list(range(NCORES)), trace=trace
    )


def _gather(results):
    cav_full = np.empty((B, S, D), np.float32)
    cat_full = np.zeros((B, S, D), np.float32)
    for c in range(NCORES):
        b, h = divmod(c, 2)
        cav_full[b, h * QH:(h + 1) * QH] = results[c]["cav"]
        cat_full[b] += results[c]["catp"]
    return cav_full, cat_full


USE_CC = False


def kernel(**inputs):
    in_maps = _prep_in_maps(**inputs, use_cc=USE_CC)
    res = run_on_device(in_maps, trace=False, use_cc=USE_CC)
    return _gather(res.results)
